# revision 9
# baseline (speedup 1.0000x reference)
"""Single-head causal attention (B=4, T=2048, C=1024) on 8 TRN2 NeuronCores.

Self-contained graded kernel: kernel(**inputs) takes FULL inputs and returns
the FULL [B, T, C] float32 output.

Key-parallel bf16 sharding (pure SPMD, no collectives): host-folded
score matrix. Since S = (Wq' x_q)·(Wk x_s) = x_s^T (Wk^T Wq') x_q, the host
precomputes M^T = Wk^T (Wq/sqrt(C)) once (constant weight preprocessing) and
the device computes G^T = M^T-contracted with the owned key columns of x —
one 65k-cycle projection replacing both the Q projection (131k) and the K
projection (65k). Per core (batch b = core//2, role r = core%2) the host
supplies x^T with T-chunks permuted into "storage order" [mine0, theirs0,
mine1, theirs1, ...], mine_j = global chunk 2j+r, so the program is
role-independent:
  - G^T[c, s] (c stationary-free dim, s = owned keys) from mt x x-even-chunks.
  - V projection of owned keys from wv.
  - Scores transposed: S^T[s, q] = sum_c G^T[c,s] x[c,q], four 512-query
    windows; chunk j joins the sweep at its birth window floor(j/2) where a
    static mask input is added (even j: [tri|thm|0|0] over the full 512;
    odd j: half-width pass over the live 256 cols with [tri|thm]; thm
    encodes the role). Garbage columns are never read by AV and are
    exp->0 for the column sums.
  - exp(S^T) tiles are the AV stationary operands directly. Row sums are
    per-window column-sum matmuls (ones stationary, 1-col LDWEIGHTS).
  - Output ships UNNORMALIZED (bf16) + row sums; the softmax division
    happens in the host combine: O = (O0u + O1u) / (l0 + l1).
"""
from contextlib import ExitStack

import numpy as np
import ml_dtypes

import concourse.tile as tile
from concourse import bacc, mybir

P = 128
B, T, C = 4, 2048, 1024
CO = C // P
N_CORES = 8
NEG = -1.0e9
HALF = T // 2
NT = T // P
NW = 4

F32 = mybir.dt.float32
BF16 = mybir.dt.bfloat16
EXP = mybir.ActivationFunctionType.Exp
COPY = mybir.ActivationFunctionType.Copy

ABLATE = set()


def _emit(nc, tc, tensors):
    (x_d, mt_d, wv_d, ma_d, mb_d, out_d, lout_d) = tensors

    with ExitStack() as ctx:
        persist = ctx.enter_context(tc.tile_pool(name="persist", bufs=1))
        g = persist.tile([P, CO, HALF], BF16, tag="g")
        v = persist.tile([P, CO, C], BF16, tag="v")
        ones = persist.tile([P, 1], BF16, tag="ones")
        pm = ctx.enter_context(tc.tile_pool(name="pm", bufs=2))
        ma = pm.tile([P, 512], BF16, tag="ma")
        mb = pm.tile([P, 256], BF16, tag="mb")

        pw = ctx.enter_context(tc.tile_pool(name="pw", bufs=1))
        mt = pw.tile([P, CO, C], BF16, tag="mt")
        wv = pw.tile([P, CO, C], BF16, tag="wv")
        # x and the masks are read until late in attention but needed at the
        # start of the next loop iteration -> double-buffer so the next
        # iteration's DMA can land during this iteration's attention
        px = ctx.enter_context(tc.tile_pool(name="px", bufs=2))
        x = px.tile([P, CO, T], BF16, tag="x")
        # even storage chunks = owned keys: x4[:, co, 0, u, :]
        x4 = x.rearrange("p co (n2 two pp) -> p co two n2 pp", two=2, pp=P)

        for co in range(CO):
            nc.sync.dma_start(wv[:, co, :512], wv_d[:, co, :512])
        for co in range(CO):
            nc.sync.dma_start(x[:, co, :P], x_d[:, co, :P])
        for co in range(CO):
            nc.sync.dma_start(x[:, co, P:HALF], x_d[:, co, P:HALF])
        for co in range(CO):
            nc.sync.dma_start(wv[:, co, 512:], wv_d[:, co, 512:])
        for co in range(CO):
            nc.sync.dma_start(mt[:, co], mt_d[:, co])
        nc.sync.dma_start(ma, ma_d)
        nc.sync.dma_start(mb, mb_d)
        for co in range(CO):
            nc.sync.dma_start(x[:, co, HALF:], x_d[:, co, HALF:])
        nc.gpsimd.memset(ones, 1.0)

        # one shared 6-bank [P,512] PSUM pool spans projections AND
        # attention (no pool-boundary drain); colsum gets its own 2 banks
        ppj = ctx.enter_context(tc.tile_pool(name="pps", bufs=7,
                                             space="PSUM"))
        ppc = ctx.enter_context(tc.tile_pool(name="ppc", bufs=1,
                                             space="PSUM"))

        # ---- G^T projection of owned keys -> g [128 c, cb, 1024 s] ----
        # G^T[c, s] = sum_c' MT[c', c] x[c', s]
        def emit_g(sw):
            for cb in range(CO):
                ps = ppj.tile([P, 512], F32, tag="ps")
                for co in range(CO):
                    nc.tensor.matmul(
                        ps, lhsT=mt[:, co, cb * P:(cb + 1) * P],
                        rhs=x4[:, co, 0, sw * 4:(sw + 1) * 4, :],
                        start=(co == 0), stop=(co == CO - 1))
                nc.vector.tensor_copy(g[:, cb, sw * 512:(sw + 1) * 512], ps)

        # ---- V projection of owned keys -> v [128 s, u, 1024] ----
        # first: its critical DMA prefix (wv half + one x chunk) is the
        # smallest, so the PE restarts fastest after the For_i barrier
        for db in range(2):
            for u in range(CO):
                ps = ppj.tile([P, 512], F32, tag="ps")
                for co in range(CO):
                    nc.tensor.matmul(
                        ps, lhsT=x4[:, co, 0, u, :],
                        rhs=wv[:, co, db * 512:(db + 1) * 512],
                        start=(co == 0), stop=(co == CO - 1))
                nc.vector.tensor_copy(v[:, u, db * 512:(db + 1) * 512], ps)

        emit_g(0)
        emit_g(1)

        # ---- attention over 4 query windows of 4 storage tiles each ----
        with tc.tile_pool(name="paw", bufs=2) as paw, \
             tc.tile_pool(name="pls", bufs=2) as pls, \
             tc.tile_pool(name="po", bufs=2) as po:
            ps_w = ps_o = ppj
            ps_c = ppc

            def emit_score_chain(w, j):
                nj = 2 * w + 2
                # odd-birth chunk (j == 2w+1): only the window's last two
                # q-tiles are live -> half-width pass over cols 256..511
                base = 256 if j == nj - 1 else 0
                wd = 512 - base
                ps = ps_w.tile([P, 512], F32, tag="ps", name="ps")[:, :wd]
                for cb in range(CO):
                    nc.tensor.matmul(
                        ps, lhsT=g[:, cb, j * P:(j + 1) * P],
                        rhs=x[:, cb, w * 512 + base:(w + 1) * 512],
                        start=(cb == 0), stop=(cb == CO - 1))
                if j == nj - 2:
                    nc.vector.tensor_add(ps, ps, ma)
                elif j == nj - 1:
                    nc.vector.tensor_add(ps, ps, mb)
                a = paw.tile([P, 512], BF16, tag=f"aw{j}", name="a")[:, :wd]
                nc.scalar.activation(a, ps, EXP)
                return (a, base)

            def emit_colsum(w, res):
                nj = 2 * w + 2
                psc = ps_c.tile([1, 512], F32, tag="psc", name="psc")
                for j in range(nj):
                    a, base = res[j]
                    nc.tensor.matmul(psc[:, base:], lhsT=ones, rhs=a,
                                     start=(j == 0), stop=(j == nj - 1))
                ls = pls.tile([1, 512], F32, tag="ls", name="ls")
                nc.vector.tensor_copy(ls, psc)
                nc.gpsimd.dma_start(lout_d[w], ls)

            def emit_av_tile(w, res, i):
                p = 4 * w + i
                n = p // 2 + 1
                ob = po.tile([P, C], BF16, tag="ob", name="ob")
                for db in range(2):
                    pso = ps_o.tile([P, 512], F32, tag="ps", name="pso")
                    for j in range(n):
                        a, base = res[j]
                        nc.tensor.matmul(
                            pso, lhsT=a[:, i * P - base:(i + 1) * P - base],
                            rhs=v[:, j, db * 512:(db + 1) * 512],
                            start=(j == 0), stop=(j == n - 1))
                    nc.scalar.activation(
                        ob[:, db * 512:(db + 1) * 512], pso, COPY)
                    nc.gpsimd.dma_start(
                        out_d[p, :, db * 512:(db + 1) * 512],
                        ob[:, db * 512:(db + 1) * 512])

            prev = None
            for w in range(NW):
                nj = 2 * w + 2
                cur = {}
                pend = list(range(4)) if prev is not None else []
                for j in range(nj):
                    cur[j] = emit_score_chain(w, j)
                    # interleave AV tiles of the previous window between
                    # score chains so PE never waits on the trailing exp
                    want = (j + 1) * 4 // nj
                    while pend and (4 - len(pend)) < want:
                        emit_av_tile(w - 1, prev, pend.pop(0))
                while pend:
                    emit_av_tile(w - 1, prev, pend.pop(0))
                emit_colsum(w, cur)
                prev = cur
            for i in range(4):
                emit_av_tile(NW - 1, prev, i)


def build(n_iters=1, ablate=(), unroll=None):
    ABLATE.clear()
    ABLATE.update(ablate)
    nc = bacc.Bacc("TRN2", target_bir_lowering=False, debug=False,
                   enable_asserts=False, num_devices=N_CORES)

    x_d = nc.dram_tensor("x", [C, T], BF16, kind="ExternalInput").ap()
    mt_d = nc.dram_tensor("mt", [C, C], BF16, kind="ExternalInput").ap()
    wv_d = nc.dram_tensor("wv", [C, C], BF16, kind="ExternalInput").ap()
    ma_d = nc.dram_tensor("ma", [P, 512], BF16, kind="ExternalInput").ap()
    mb_d = nc.dram_tensor("mb", [P, 256], BF16, kind="ExternalInput").ap()
    out_d = nc.dram_tensor("out", [NT, P, C], BF16, kind="ExternalOutput").ap()
    lout_d = nc.dram_tensor("lsum", [NW, 1, 512], F32,
                            kind="ExternalOutput").ap()

    def r(ap):
        return ap.rearrange("(co cp) s -> cp co s", cp=P)

    tensors = (r(x_d), r(mt_d), r(wv_d), ma_d, mb_d, out_d, lout_d)

    with tile.TileContext(nc) as tc:
        if n_iters > 1:
            u = unroll or next(
                u for u in (32, 16, 8, 4, 2, 1) if n_iters % u == 0)
            assert n_iters % u == 0
            with tc.For_i(0, n_iters // u):
                for _ in range(u):
                    _emit(nc, tc, tensors)
        else:
            _emit(nc, tc, tensors)

    nc.compile()
    return nc


def _bf(a):
    return np.asarray(a, np.float32).astype(ml_dtypes.bfloat16)


def make_in_maps(input_x, Wq, Wk, Wv):
    scale = np.float32(C) ** -0.5
    MT = (np.asarray(Wk, np.float64).T
          @ (np.asarray(Wq, np.float64) * scale)).astype(np.float32)
    mt = _bf(MT)
    wv = _bf(np.ascontiguousarray(np.asarray(Wv).T))
    s_i = np.arange(P)[:, None]
    q_i = np.arange(P)[None, :]
    tri = np.where(s_i <= q_i, 0.0, NEG).astype(np.float32)
    zero = np.zeros((P, P), np.float32)
    neg = np.full((P, P), NEG, np.float32)
    in_maps = []
    for core in range(N_CORES):
        b, role = divmod(core, 2)
        thm = zero if role == 0 else neg
        maskA = np.concatenate([tri, thm, zero, zero], axis=1)
        maskB = np.concatenate([tri, thm], axis=1)
        xTb = _bf(np.ascontiguousarray(np.asarray(input_x[b]).T))
        if role == 1:
            xc = xTb.reshape(C, NT, P)
            perm = [gg ^ 1 for gg in range(NT)]
            xTb = np.ascontiguousarray(xc[:, perm, :].reshape(C, T))
        in_maps.append({"x": xTb, "mt": mt, "wv": wv,
                        "ma": _bf(maskA), "mb": _bf(maskB)})
    return in_maps


def unshard(results):
    out = np.empty((B, T, C), np.float32)
    for b in range(B):
        o = [results[2 * b + r]["out"].astype(np.float32) for r in range(2)]
        ls = [results[2 * b + r]["lsum"].astype(np.float32).reshape(T)
              for r in range(2)]
        for gg in range(NT):
            p0, p1 = gg, gg ^ 1  # storage slot of global tile gg on role 0/1
            l = (ls[0][p0 * P:(p0 + 1) * P] + ls[1][p1 * P:(p1 + 1) * P])
            out[b, gg * P:(gg + 1) * P] = (
                o[0][p0] + o[1][p1]) / l[:, None]
    return out


_CACHED_NC = None


def kernel(input_x, Wq, Wk, Wv):
    global _CACHED_NC
    if _CACHED_NC is None:
        _CACHED_NC = build(n_iters=1)
    nc = _CACHED_NC

    in_maps = make_in_maps(input_x, Wq, Wk, Wv)
    from concourse import bass_utils
    res = bass_utils.run_bass_kernel_spmd(
        nc, in_maps, core_ids=list(range(N_CORES)))
    return unshard(res.results)
